# revision 43
# baseline (speedup 1.0000x reference)
"""Trainium2 Bass kernel for the CAM-threshold-subtract module.

Computation (per sample b):
    idx    = argmax(logits[b, :])                 # over 1000 classes
    cam    = interm[b, :, :, idx]                 # [7,7] gather
    t      = where(cam > 0.5, cam, 0)
    out[b] = vgg[b] - broadcast(t, [7,7,512])

Sharding: pure data parallel, batch 256 -> 8 cores x 32 samples.

Per-core memory traffic is dominated by vgg (3.2MB read) + out (3.2MB
write); interm is NOT streamed - only 49 floats per sample are fetched
with one indirect-DMA gather (32 descriptors), using a combined index
b*1000 + idx into a [32,1000,49]-strided logical view of interm.
"""

import numpy as np

M = 8          # cores
B = 32         # samples per core
S = 49         # spatial positions (7*7)
C = 512        # vgg channels
K = 1000       # classes
P = 128        # partitions
ROWS = B * S   # 1568 (b,pos) rows per core
NFULL = ROWS // P          # 12 full [128,512] tiles
REM = ROWS - NFULL * P     # 32 rows in the last tile
NT = NFULL + 1             # 13
THRESH = 0.5
FREE_LOADS = 6  # loads issued immediately; the rest wait for the CAM gather


def _build(loop_n=None, do_t=True, do_load=True, do_sub=True, do_store=True,
           sub_dummy=False, t_depth=4):
    import contextlib

    import concourse.bacc as bacc
    import concourse.bass as bass
    import concourse.tile as tile
    from concourse import mybir

    nc = bacc.Bacc("TRN2", target_bir_lowering=False, debug=False)
    vgg = nc.dram_tensor("vgg", [ROWS, C], mybir.dt.float32, kind="ExternalInput")
    # interm is pre-transposed on host to [B, K, S] so each CAM row
    # (one channel's 49 spatial values) is contiguous for the row-gather.
    interm = nc.dram_tensor("interm", [B, K, S], mybir.dt.float32, kind="ExternalInput")
    logits = nc.dram_tensor("logits", [B, K], mybir.dt.float32, kind="ExternalInput")
    out = nc.dram_tensor("out", [ROWS, C], mybir.dt.float32, kind="ExternalOutput")

    with tile.TileContext(nc) as tc:
        with (
            tc.tile_pool(name="big", bufs=NT) as big,
            tc.tile_pool(name="small", bufs=1) as small,
            tc.tile_pool(name="dram", bufs=1, space="DRAM") as dpool,
            tc.For_i(0, loop_n) if loop_n else contextlib.nullcontext(),
        ):
            if do_t:
                # The t-path's small DMAs live on the scalar/gpsimd rings so
                # they never queue behind the 13 big vgg loads (sync ring is
                # FIFO per engine).
                # ---- per-sample argmax over class logits ----
                lg = small.tile([B, K], mybir.dt.float32)
                nc.scalar.dma_start(out=lg[:], in_=logits.ap()[:, :])
                mx = small.tile([B, 8], mybir.dt.float32)
                nc.vector.max(mx[:], lg[:])
                mi = small.tile([B, 8], mybir.dt.uint32)
                nc.vector.max_index(mi[:], mx[:], lg[:])

                # combined row index into interm viewed [B*K, S]: b*1000 + idx_b
                base = small.tile([B, 1], mybir.dt.uint32)
                nc.gpsimd.iota(base[:], [[1, 1]], base=0, channel_multiplier=K)
                comb = small.tile([B, 1], mybir.dt.uint32)
                nc.vector.tensor_tensor(
                    out=comb[:], in0=mi[:, 0:1], in1=base[:], op=mybir.AluOpType.add
                )

                gather_inst = None
                if t_depth >= 2:
                    # ---- row-gather the CAM: cam[b,:] = interm[b, idx_b, :] ----
                    # one descriptor per sample, 49 contiguous floats each
                    cam = small.tile([B, S], mybir.dt.float32)
                    gather_inst = nc.gpsimd.indirect_dma_start(
                        out=cam[:],
                        out_offset=None,
                        in_=interm.ap().rearrange("b k s -> (b k) s"),
                        in_offset=bass.IndirectOffsetOnAxis(
                            ap=comb[:, 0:1], axis=0
                        ),
                    )

                if t_depth >= 3:
                    # ---- threshold: t = cam * (cam > 0.5) ----
                    mask = small.tile([B, S], mybir.dt.float32)
                    nc.vector.tensor_scalar(
                        out=mask[:], in0=cam[:], scalar1=THRESH, scalar2=None,
                        op0=mybir.AluOpType.is_gt,
                    )
                    tt = small.tile([B, S], mybir.dt.float32)
                    nc.vector.tensor_tensor(
                        out=tt[:], in0=cam[:], in1=mask[:],
                        op=mybir.AluOpType.mult,
                    )
                    # ---- refold t [32,49] -> [128,13] via DRAM bounce ----
                    # (row g = b*49+pos; tile k holds rows 128k..128k+127)
                    td = dpool.tile([NT, P], mybir.dt.float32)  # flat [1664]
                    nc.gpsimd.dma_start(
                        out=td[:].flatten()[0:ROWS].rearrange("(b s) -> b s", b=B),
                        in_=tt[:],
                    )

                if t_depth >= 4:
                    # reload split in two so no pad elements are ever touched:
                    # [128,12] strided main block + [32,1] corner of tile 12
                    t_all = small.tile([P, NT], mybir.dt.float32)
                    nc.scalar.dma_start(
                        out=t_all[:, 0:NFULL], in_=td[0:NFULL, :].transpose([1, 0])
                    )
                    nc.gpsimd.dma_start(
                        out=t_all[0:REM, NFULL:NT],
                        in_=td[NFULL:NT, 0:REM].transpose([1, 0]),
                    )
                else:
                    t_all = small.tile([P, NT], mybir.dt.float32)
                    nc.vector.memset(t_all[:], 0.0)
            else:
                gather_inst = None
                t_all = small.tile([P, NT], mybir.dt.float32)
                nc.vector.memset(t_all[:], 0.0)
            if sub_dummy:
                t_all = small.tile([P, NT], mybir.dt.float32, tag="t_dummy")
                nc.vector.memset(t_all[:], 0.0)

            # ---- main stream: out = vgg - t (per-row scalar broadcast) ----
            for k in range(NT if (do_load or do_sub or do_store) else 0):
                rows = P if k < NFULL else REM
                vt = big.tile([P, C], mybir.dt.float32, tag="vt")
                if do_load:
                    ld = nc.sync.dma_start(
                        out=vt[:rows, :], in_=vgg.ap()[k * P : k * P + rows, :]
                    )
                    if gather_inst is not None and k >= FREE_LOADS:
                        # hold back later loads so the t-chain's small DMAs
                        # complete at idle-HBM latency instead of queueing
                        # behind 3MB of load descriptors
                        bass._add_dep_helper(
                            ld.ins, gather_inst.ins, sync=True,
                            reason="stagger loads behind CAM gather",
                        )
                else:
                    nc.vector.memset(vt[:rows, :], 0.0)
                if do_sub:
                    nc.vector.tensor_scalar(
                        out=vt[:rows, :], in0=vt[:rows, :],
                        scalar1=t_all[:rows, k : k + 1], scalar2=None,
                        op0=mybir.AluOpType.subtract,
                    )
                if do_store:
                    nc.scalar.dma_start(
                        out=out.ap()[k * P : k * P + rows, :], in_=vt[:rows, :]
                    )
    nc.compile()
    return nc


def _build_v2(loop_n=None, free_loads=99, sim_safe=False,
              do_t=True, do_load=True, do_sub=True, do_store=True):
    """Bounce-free design.

    16 dense tiles of [98, 512], tile k = samples {2k, 2k+1} (partition
    49*b2 + s).  The CAM fold is done on-chip: PE-transpose [32,49] ->
    PSUM [49,32], threshold, then two strided DVE copies build the
    [98,16] per-partition scalar table.  The t-chain has only two DMA
    links (logits load, CAM gather).
    """
    import contextlib

    import concourse.bacc as bacc
    import concourse.bass as bass
    import concourse.tile as tile
    from concourse import mybir
    from concourse.masks import make_identity

    KT = B // 2      # 16 tiles, 2 samples each
    SP = 64          # sample B's rows sit at partition base 64 (HW-aligned)
    RV = SP + S      # 113 partitions carry data (rows 49..63 are filler)

    nc = bacc.Bacc("TRN2", target_bir_lowering=False, debug=False)
    vgg = nc.dram_tensor("vgg", [ROWS, C], mybir.dt.float32, kind="ExternalInput")
    interm = nc.dram_tensor("interm", [B, K, S], mybir.dt.float32, kind="ExternalInput")
    logits = nc.dram_tensor("logits", [B, K], mybir.dt.float32, kind="ExternalInput")
    out = nc.dram_tensor("out", [ROWS, C], mybir.dt.float32, kind="ExternalOutput")

    with tile.TileContext(nc) as tc:
        with (
            tc.tile_pool(name="big", bufs=KT) as big,
            tc.tile_pool(name="small", bufs=1) as small,
            tc.tile_pool(name="psum", bufs=1, space="PSUM") as psum,
            tc.For_i(0, loop_n) if loop_n else contextlib.nullcontext(),
        ):
            if do_t:
                ident = small.tile([B, B], mybir.dt.float32)
                make_identity(nc, ident[:])

                # ---- per-sample argmax over class logits ----
                lg = small.tile([B, K], mybir.dt.float32)
                nc.scalar.dma_start(out=lg[:], in_=logits.ap()[:, :])
                mx = small.tile([B, 8], mybir.dt.float32)
                nc.vector.max(mx[:], lg[:])
                mi = small.tile([B, 8], mybir.dt.uint32)
                nc.vector.max_index(mi[:], mx[:], lg[:])
                base = small.tile([B, 1], mybir.dt.uint32)
                nc.gpsimd.iota(base[:], [[1, 1]], base=0, channel_multiplier=K)
                comb = small.tile([B, 1], mybir.dt.uint32)
                nc.vector.tensor_tensor(
                    out=comb[:], in0=mi[:, 0:1], in1=base[:],
                    op=mybir.AluOpType.add,
                )

                # ---- row-gather the CAM: cam[b,:] = interm[b, idx_b, :] ----
                cam = small.tile([B, S], mybir.dt.float32)
                gather_inst = nc.gpsimd.indirect_dma_start(
                    out=cam[:],
                    out_offset=None,
                    in_=interm.ap().rearrange("b k s -> (b k) s"),
                    in_offset=bass.IndirectOffsetOnAxis(ap=comb[:, 0:1], axis=0),
                )

                # ---- threshold, then fold on-chip ----
                mask = small.tile([B, S], mybir.dt.float32)
                nc.vector.tensor_scalar(
                    out=mask[:], in0=cam[:], scalar1=THRESH, scalar2=None,
                    op0=mybir.AluOpType.is_gt,
                )
                ttv = small.tile([B, S], mybir.dt.float32)
                nc.vector.tensor_tensor(
                    out=ttv[:], in0=cam[:], in1=mask[:], op=mybir.AluOpType.mult
                )
                # PE transpose: [32, 49] -> PSUM [49, 32]
                pt = psum.tile([S, B], mybir.dt.float32)
                nc.tensor.transpose(pt[:], ttv[:], ident[:])
                # scalar table ct[p, k]: rows 0..48 = t[2k], rows 64..112 =
                # t[2k+1]; filler rows 49..63 zeroed (32-aligned window)
                ct = small.tile([P, KT], mybir.dt.float32)
                nc.vector.memset(ct[32:SP, :], 0.0)
                nc.vector.tensor_copy(out=ct[0:S, :], in_=pt[:, 0:B:2])
                nc.vector.tensor_copy(out=ct[SP:RV, :], in_=pt[:, 1:B:2])
            else:
                gather_inst = None
                ct = small.tile([P, KT], mybir.dt.float32)
                nc.vector.memset(ct[:], 0.0)

            # ---- main stream ----
            vgg3 = vgg.ap().rearrange("(b s) c -> b s c", s=S)
            out3 = out.ap().rearrange("(b s) c -> b s c", s=S)
            for k in range(KT if (do_load or do_sub or do_store) else 0):
                vt = big.tile([P, C], mybir.dt.float32, tag="vt")
                if do_load:
                    for b2 in range(2):
                        ld = nc.sync.dma_start(
                            out=vt[SP * b2 : SP * b2 + S, :],
                            in_=vgg3[2 * k + b2, :, :],
                        )
                        if gather_inst is not None and k >= free_loads:
                            bass._add_dep_helper(
                                ld.ins, gather_inst.ins, sync=True,
                                reason="stagger loads behind CAM gather",
                            )
                else:
                    nc.vector.memset(vt[:], 0.0)
                if not do_sub:
                    pass
                elif sim_safe:
                    # CoreSim refuses reads of uninitialized SBUF, so split
                    # the subtract over the two valid partition windows
                    for b2 in range(2):
                        nc.vector.tensor_scalar(
                            out=vt[SP * b2 : SP * b2 + S, :],
                            in0=vt[SP * b2 : SP * b2 + S, :],
                            scalar1=ct[SP * b2 : SP * b2 + S, k : k + 1],
                            scalar2=None,
                            op0=mybir.AluOpType.subtract,
                        )
                else:
                    # single op over partitions 0..112; filler rows 49..63
                    # hold junk that is computed on but never stored
                    nc.vector.tensor_scalar(
                        out=vt[0:RV, :], in0=vt[0:RV, :],
                        scalar1=ct[0:RV, k : k + 1], scalar2=None,
                        op0=mybir.AluOpType.subtract,
                    )
                if do_store:
                    for b2 in range(2):
                        nc.scalar.dma_start(
                            out=out3[2 * k + b2, :, :],
                            in_=vt[SP * b2 : SP * b2 + S, :],
                        )
    nc.compile()
    return nc


def _build_v3(loop_n=None, free_loads=99,
              do_t=True, do_load=True, do_sub=True, do_store=True):
    """Dense-tile design with an on-chip CAM fold.

    16 dense tiles of [98, 512] (tile k = samples {2k, 2k+1}), one
    contiguous load/store DMA each.  t-chain: logits load -> argmax ->
    row-gather CAM [32,49] -> threshold -> PE transpose to PSUM [49,32]
    -> even half of the scalar table via a strided DVE copy (base 0),
    odd half via a small SBUF->SBUF DMA to partition base 49.
    """
    import contextlib

    import concourse.bacc as bacc
    import concourse.bass as bass
    import concourse.tile as tile
    from concourse import mybir
    from concourse.masks import make_identity

    KT = B // 2      # 16 tiles, 2 samples each
    RT = 2 * S       # 98 rows per tile

    nc = bacc.Bacc("TRN2", target_bir_lowering=False, debug=False)
    vgg = nc.dram_tensor("vgg", [ROWS, C], mybir.dt.float32, kind="ExternalInput")
    interm = nc.dram_tensor("interm", [B, K, S], mybir.dt.float32, kind="ExternalInput")
    logits = nc.dram_tensor("logits", [B, K], mybir.dt.float32, kind="ExternalInput")
    out = nc.dram_tensor("out", [ROWS, C], mybir.dt.float32, kind="ExternalOutput")

    with tile.TileContext(nc) as tc:
        with (
            tc.tile_pool(name="big", bufs=KT) as big,
            tc.tile_pool(name="small", bufs=1) as small,
            tc.tile_pool(name="psum", bufs=1, space="PSUM") as psum,
            tc.For_i(0, loop_n) if loop_n else contextlib.nullcontext(),
        ):
            if do_t:
                ident = small.tile([B, B], mybir.dt.float32)
                make_identity(nc, ident[:])

                lg = small.tile([B, K], mybir.dt.float32)
                nc.scalar.dma_start(out=lg[:], in_=logits.ap()[:, :])
                mx = small.tile([B, 8], mybir.dt.float32)
                nc.vector.max(mx[:], lg[:])
                mi = small.tile([B, 8], mybir.dt.uint32)
                nc.vector.max_index(mi[:], mx[:], lg[:])
                base = small.tile([B, 1], mybir.dt.uint32)
                nc.gpsimd.iota(base[:], [[1, 1]], base=0, channel_multiplier=K)
                comb = small.tile([B, 1], mybir.dt.uint32)
                nc.vector.tensor_tensor(
                    out=comb[:], in0=mi[:, 0:1], in1=base[:],
                    op=mybir.AluOpType.add,
                )

                cam = small.tile([B, S], mybir.dt.float32)
                gather_inst = nc.gpsimd.indirect_dma_start(
                    out=cam[:],
                    out_offset=None,
                    in_=interm.ap().rearrange("b k s -> (b k) s"),
                    in_offset=bass.IndirectOffsetOnAxis(ap=comb[:, 0:1], axis=0),
                )

                mask = small.tile([B, S], mybir.dt.float32)
                nc.vector.tensor_scalar(
                    out=mask[:], in0=cam[:], scalar1=THRESH, scalar2=None,
                    op0=mybir.AluOpType.is_gt,
                )
                ttv = small.tile([B, S], mybir.dt.float32)
                nc.vector.tensor_tensor(
                    out=ttv[:], in0=cam[:], in1=mask[:], op=mybir.AluOpType.mult
                )
                pt = psum.tile([S, B], mybir.dt.float32)
                nc.tensor.transpose(pt[:], ttv[:], ident[:])

                # dense scalar table ct[49*b2+s, k] = t[2k+b2, s]
                ct = small.tile([RT, KT], mybir.dt.float32)
                nc.vector.tensor_copy(out=ct[0:S, :], in_=pt[:, 0:B:2])
                podd = small.tile([S, KT], mybir.dt.float32)
                nc.vector.tensor_copy(out=podd[:], in_=pt[:, 1:B:2])
                nc.gpsimd.dma_start(out=ct[S:RT, :], in_=podd[:])
            else:
                gather_inst = None
                ct = small.tile([RT, KT], mybir.dt.float32)
                nc.vector.memset(ct[:], 0.0)

            # ---- main stream: dense [98, 512] tiles ----
            for k in range(KT if (do_load or do_sub or do_store) else 0):
                vt = big.tile([RT, C], mybir.dt.float32, tag="vt")
                if do_load:
                    ld = nc.sync.dma_start(
                        out=vt[:], in_=vgg.ap()[k * RT : (k + 1) * RT, :]
                    )
                    if gather_inst is not None and k >= free_loads:
                        bass._add_dep_helper(
                            ld.ins, gather_inst.ins, sync=True,
                            reason="stagger loads behind CAM gather",
                        )
                else:
                    nc.vector.memset(vt[:], 0.0)
                if do_sub:
                    nc.vector.tensor_scalar(
                        out=vt[:], in0=vt[:], scalar1=ct[:, k : k + 1],
                        scalar2=None, op0=mybir.AluOpType.subtract,
                    )
                if do_store:
                    nc.scalar.dma_start(
                        out=out.ap()[k * RT : (k + 1) * RT, :], in_=vt[:]
                    )
    nc.compile()
    return nc


def _consts_np():
    """Selection masks for the on-PE CAM fold.

    Column j of the table corresponds to flat row g=j (tile j//128,
    partition j%128).  BSEL[b, j] = 1 iff sample b owns row j;
    SELMASK[s, j] = 1 iff position s matches row j.  Columns j >= 1568
    are zero (tile 12 pad), making the folded values there exactly 0.
    """
    j = np.arange(NT * P)
    valid = j < ROWS
    bsel = (j // S == np.arange(B)[:, None]) & valid
    smask = (j % S == np.arange(S)[:, None]) & valid
    return np.concatenate([bsel, smask], 0).astype(np.float32)  # [81, 1664]


def _build_v6(loop_n=None,
              do_t=True, do_load=True, do_sub=True, do_store=True):
    """Dense stream + on-PE CAM fold via constant selection masks.

    Stream: 13 dense [128,512] tiles (line-rate DMA).  t-chain: logits
    load -> argmax -> row-gather CAM [32,49] -> threshold -> P1 =
    tt^T @ BSEL (4 matmuls) -> mask-mult by SELMASK -> column-sum
    matmuls -> ct [128,13] per-partition scalars.  Only two DMA links
    (logits, gather) on the critical chain; everything partition-aligned.
    """
    import contextlib

    import concourse.bacc as bacc
    import concourse.bass as bass
    import concourse.tile as tile
    from concourse import mybir

    W = NT * P  # 1664 table columns

    nc = bacc.Bacc("TRN2", target_bir_lowering=False, debug=False)
    vgg = nc.dram_tensor("vgg", [ROWS, C], mybir.dt.float32, kind="ExternalInput")
    interm = nc.dram_tensor("interm", [B, K, S], mybir.dt.float32, kind="ExternalInput")
    logits = nc.dram_tensor("logits", [B, K], mybir.dt.float32, kind="ExternalInput")
    consts = nc.dram_tensor("consts", [B + S, W], mybir.dt.float32, kind="ExternalInput")
    out = nc.dram_tensor("out", [ROWS, C], mybir.dt.float32, kind="ExternalOutput")

    with tile.TileContext(nc) as tc:
        with (
            tc.tile_pool(name="big", bufs=NT) as big,
            tc.tile_pool(name="small", bufs=1) as small,
            tc.tile_pool(name="psum", bufs=1, space="PSUM") as psum,
            tc.For_i(0, loop_n) if loop_n else contextlib.nullcontext(),
        ):
            if do_t:
                # selection masks + all-ones vector (off the critical chain)
                bsel = small.tile([B, W], mybir.dt.float32)
                nc.scalar.dma_start(out=bsel[:], in_=consts.ap()[0:B, :])
                smask = small.tile([S, W], mybir.dt.float32)
                nc.scalar.dma_start(out=smask[:], in_=consts.ap()[B : B + S, :])
                ones = small.tile([S, 1], mybir.dt.float32)
                nc.gpsimd.memset(ones[:], 1.0)

                # logits first on the sync ring, ahead of the vgg loads
                lg = small.tile([B, K], mybir.dt.float32)
                nc.sync.dma_start(out=lg[:], in_=logits.ap()[:, :])
                mx = small.tile([B, 8], mybir.dt.float32)
                nc.vector.max(mx[:], lg[:])
                mi = small.tile([B, 8], mybir.dt.uint32)
                nc.vector.max_index(mi[:], mx[:], lg[:])
                base = small.tile([B, 1], mybir.dt.uint32)
                nc.gpsimd.iota(base[:], [[1, 1]], base=0, channel_multiplier=K)
                comb = small.tile([B, 1], mybir.dt.uint32)
                nc.vector.tensor_tensor(
                    out=comb[:], in0=mi[:, 0:1], in1=base[:],
                    op=mybir.AluOpType.add,
                )

                cam = small.tile([B, S], mybir.dt.float32)
                nc.gpsimd.indirect_dma_start(
                    out=cam[:],
                    out_offset=None,
                    in_=interm.ap().rearrange("b k s -> (b k) s"),
                    in_offset=bass.IndirectOffsetOnAxis(ap=comb[:, 0:1], axis=0),
                )

                mask = small.tile([B, S], mybir.dt.float32)
                nc.vector.tensor_scalar(
                    out=mask[:], in0=cam[:], scalar1=THRESH, scalar2=None,
                    op0=mybir.AluOpType.is_gt,
                )
                ttv = small.tile([B, S], mybir.dt.float32)
                nc.vector.tensor_tensor(
                    out=ttv[:], in0=cam[:], in1=mask[:], op=mybir.AluOpType.mult
                )

                # P1[s, j] = t[b(j), s]  (one-hot matmul over samples)
                p1 = psum.tile([S, W], mybir.dt.float32)
                for q in range(0, W, 512):
                    n = min(512, W - q)
                    nc.tensor.matmul(
                        out=p1[:, q : q + n], lhsT=ttv[:],
                        rhs=bsel[:, q : q + n], start=True, stop=True,
                    )
                # keep only s = s(j), then column-sum -> ct[p, T] = t[g]
                l2 = small.tile([S, W], mybir.dt.float32)
                nc.vector.tensor_tensor(
                    out=l2[:], in0=p1[:], in1=smask[:],
                    op=mybir.AluOpType.mult,
                )
                ctp = psum.tile([P, NT], mybir.dt.float32)
                for T in range(NT):
                    nc.tensor.matmul(
                        out=ctp[:, T : T + 1],
                        lhsT=l2[:, T * P : (T + 1) * P],
                        rhs=ones[:], start=True, stop=True,
                    )
                ct = small.tile([P, NT], mybir.dt.float32)
                nc.vector.tensor_copy(out=ct[:], in_=ctp[:])
            else:
                ct = small.tile([P, NT], mybir.dt.float32)
                nc.vector.memset(ct[:], 0.0)

            # ---- main stream: 13 dense [128, 512] tiles ----
            for k in range(NT if (do_load or do_sub or do_store) else 0):
                rows = P if k < NFULL else REM
                vt = big.tile([P, C], mybir.dt.float32, tag="vt")
                if do_load:
                    nc.sync.dma_start(
                        out=vt[:rows, :], in_=vgg.ap()[k * P : k * P + rows, :]
                    )
                else:
                    nc.vector.memset(vt[:rows, :], 0.0)
                if do_sub:
                    nc.vector.tensor_scalar(
                        out=vt[:rows, :], in0=vt[:rows, :],
                        scalar1=ct[:rows, k : k + 1], scalar2=None,
                        op0=mybir.AluOpType.subtract,
                    )
                if do_store:
                    nc.scalar.dma_start(
                        out=out.ap()[k * P : k * P + rows, :], in_=vt[:rows, :]
                    )
    nc.compile()
    return nc


_NC = None


def _get_nc():
    global _NC
    if _NC is None:
        _NC = _build()
    return _NC


def _shard(vgg_end, interm, branchA_end):
    consts = _consts_np()
    in_maps = []
    for i in range(M):
        sl = slice(i * B, (i + 1) * B)
        in_maps.append(
            {
                "vgg": np.ascontiguousarray(vgg_end[sl], dtype=np.float32).reshape(ROWS, C),
                "interm": np.ascontiguousarray(
                    np.asarray(interm[sl], dtype=np.float32).reshape(B, S, K).transpose(0, 2, 1)
                ),
                "logits": np.ascontiguousarray(branchA_end[sl], dtype=np.float32),
                "consts": consts,
            }
        )
    return in_maps


def kernel(vgg_end, interm, branchA_end):
    from concourse.bass_utils import run_bass_kernel_spmd

    nc = _get_nc()
    in_maps = _shard(np.asarray(vgg_end), np.asarray(interm), np.asarray(branchA_end))
    res = run_bass_kernel_spmd(nc, in_maps, core_ids=list(range(M)))
    return np.concatenate(
        [r["out"].reshape(B, 7, 7, C) for r in res.results], axis=0
    )
